# revision 27
# baseline (speedup 1.0000x reference)
"""GCN layer (x@W, sparse-adj aggregate, +bias) on 8 Trainium2 NeuronCores.

Strategy (memory-regime), aggregate-then-project:
  out = A @ (x @ W) + b == (A @ x) @ W + b

  - Destination nodes sharded 12500/core (1D graph partition per hint).
  - Edges bucketed by (window-batch of gw dest windows, int16 index group
    of 32768 source rows), sorted by dest window within each bucket,
    padded to 128-slot tiles (host-side, shared tile counts across cores).
  - The gather reads RAW x rows (256B bf16) straight from the input
    table in DRAM via batched dma_gather ucode calls (<=512 idxs/call,
    round-robin over 4 SWDGE queues, 32KB descriptor scratch) -- no
    projection table is materialized, so gathering starts immediately.
  - Gathered rows are scaled in place by edge values (broadcast
    tensor_tensor); one-hot scatter matrices S for chunks of matmuls are
    built with broadcast is_equal tensor_tensor ops; PE matmuls
    G_t.T @ S accumulate agg^T per 128-row dest window in PSUM.
  - agg^T windows are evacuated to SBUF bf16 and projected by W
    (lhsT=W, rhs=agg^T) into out^T; bias is added per-partition on PSUM
    evacuation; one streaming DMA per batch writes out^T; the host
    transposes back.
"""

import math
import sys

import numpy as np

for _p in ("/opt/trn_rl_repo",):
    if _p not in sys.path:
        sys.path.insert(0, _p)

import ml_dtypes  # noqa: E402

from concourse import bacc, bass, mybir, tile  # noqa: E402
from concourse import bass_utils  # noqa: E402

BF16 = mybir.dt.bfloat16
F32 = mybir.dt.float32
I16 = mybir.dt.int16
NP_BF16 = ml_dtypes.bfloat16

P = 128
GROUP_ROWS = 32768  # int16 index reach of dma_gather
CALL_TILES = 4  # ucode caps one dma_gather at 512 idxs = 4 tiles
RL_PAD = 255.0  # rloc sentinel that never matches iota (0..127)


def default_cfg():
    return dict(
        n_nodes=100000,
        n_edges=800000,
        in_f=128,
        out_f=64,
        n_cores=8,
        gw=12,  # dest windows per batch (PSUM: 12*512B = 3 banks per buf)
        sc=32,  # matmuls per S-matrix build chunk
        nq=4,  # SWDGE queues for gather descriptor rings
    )


def _derived(cfg):
    n_nodes = cfg["n_nodes"]
    c = cfg["n_cores"]
    ns = n_nodes // c  # dest rows per core
    nw = math.ceil(ns / P)  # dest windows per core
    ntab = math.ceil(n_nodes / P)  # source row tiles
    npad = ntab * P
    return ns, nw, ntab, npad


def prep_inputs(x, weights, bias, adj_rows, adj_cols, adj_vals, cfg):
    """Host-side sharding/index prep (numpy only). Returns (in_maps, shared)."""
    c = cfg["n_cores"]
    in_f = cfg["in_f"]
    gw = cfg["gw"]
    ns, nw, ntab, npad = _derived(cfg)
    nb = math.ceil(nw / gw)
    ngrp = math.ceil(npad / GROUP_ROWS)

    x = np.asarray(x, dtype=np.float32)
    weights = np.asarray(weights, dtype=np.float32)
    bias = np.asarray(bias, dtype=np.float32)
    rows = np.asarray(adj_rows).astype(np.int64)
    cols = np.asarray(adj_cols).astype(np.int64)
    vals = np.asarray(adj_vals, dtype=np.float32)

    xb = np.zeros((npad, in_f), dtype=NP_BF16)
    xb[: x.shape[0]] = x.astype(NP_BF16)
    wt = weights.astype(NP_BF16)
    biasT = np.ascontiguousarray(bias[:, None]).astype(np.float32)  # [64, 1]
    iota = np.broadcast_to(np.arange(P, dtype=np.float32), (P, P)).astype(NP_BF16)
    iota = np.ascontiguousarray(iota)

    # per-edge attributes
    core = rows // ns
    rloc = rows - core * ns
    w = rloc // P
    d = rloc % P
    b = w // gw
    g = cols // GROUP_ROWS

    # sort by (core, batch, group, window)
    key = ((core * nb + b) * ngrp + g) * nw + w
    order = np.argsort(key, kind="stable")
    core_s = core[order]
    b_s = b[order]
    g_s = g[order]
    w_s = w[order]
    d_s = d[order]
    col_s = cols[order]
    vv_s = vals[order]

    # bucket counts n[core, b, g]
    bg = b_s * ngrp + g_s
    cnt = np.zeros((c, nb * ngrp), dtype=np.int64)
    for ci in range(c):
        m = core_s == ci
        cnt[ci] = np.bincount(bg[m], minlength=nb * ngrp)
    cnt = cnt.reshape(c, nb, ngrp)
    T = -(-cnt.max(axis=0) // P)  # [nb, ngrp] shared tile counts

    tile_base = np.zeros((nb, ngrp), dtype=np.int64)
    np.cumsum(T.ravel()[:-1], out=tile_base.ravel()[1:])
    ntile_tot = int(T.sum())
    batch_tiles = T.sum(axis=1)  # tiles per batch
    batch_tile0 = np.concatenate([[0], np.cumsum(batch_tiles)[:-1]])

    # per-core slot assignment within each (b, g) bucket
    core_start = np.searchsorted(core_s, np.arange(c + 1))
    slot = np.zeros(len(order), dtype=np.int64)  # local slot within bucket
    for ci in range(c):
        s, e = core_start[ci], core_start[ci + 1]
        bgl = bg[s:e]
        bstart = np.searchsorted(bgl, np.arange(nb * ngrp))
        slot[s:e] = np.arange(e - s) - bstart[bgl]

    # shared matmul schedule: per (b, g, tile): union window span over cores
    lo = np.full((ntile_tot,), 1 << 30, dtype=np.int64)
    hi = np.full((ntile_tot,), -1, dtype=np.int64)
    gt_all = tile_base[b_s, g_s] + slot // P  # global tile per edge
    np.minimum.at(lo, gt_all, w_s)
    np.maximum.at(hi, gt_all, w_s)
    # tiles with no edges anywhere: one dummy matmul at the batch's first
    # window (S will be all-zero)
    for b_i in range(nb):
        for g_i in range(ngrp):
            for t_l in range(T[b_i, g_i]):
                gt = tile_base[b_i, g_i] + t_l
                if hi[gt] < 0:
                    lo[gt] = hi[gt] = b_i * gw
    span = hi - lo + 1
    # presence[gt, wv-lo]: does ANY core have an edge in (tile, window)?
    # Scheduling only present pairs trims union-span middles with no edges.
    presence = np.zeros((ntile_tot, int(span.max())), dtype=bool)
    presence[gt_all, w_s - lo[gt_all]] = True
    for b_i in range(nb):
        for g_i in range(ngrp):
            for t_l in range(T[b_i, g_i]):
                gt = tile_base[b_i, g_i] + t_l
                if not presence[gt, : span[gt]].any():
                    presence[gt, 0] = True  # dummy (all-zero S)

    # per-batch schedule entries (tile_in_batch, wrel, start, stop), ordered
    # window-major so only ONE PSUM accumulation group is open at a time
    # (PSUM groups conflict at bank granularity). col_map[(gt, wv-lo)] gives
    # each edge its rl column under the final order.
    col_map = np.zeros((ntile_tot, int(span.max())), dtype=np.int64)
    sched = []
    mm_base = []
    col = 0
    for b_i in range(nb):
        raw = []
        for g_i in range(ngrp):
            for t_l in range(T[b_i, g_i]):
                gt = tile_base[b_i, g_i] + t_l
                for wv in range(lo[gt], hi[gt] + 1):
                    if presence[gt, wv - lo[gt]]:
                        raw.append((wv - b_i * gw, gt))
        raw.sort()
        entries = []
        gwb = min(gw, nw - b_i * gw)
        seen = set()
        for i, (wrel, gt) in enumerate(raw):
            first = wrel not in seen
            seen.add(wrel)
            last = i + 1 == len(raw) or raw[i + 1][0] != wrel
            entries.append([gt - batch_tile0[b_i], wrel, first, last])
            col_map[gt, wrel + b_i * gw - lo[gt]] = col + i
        for wrel in range(gwb):
            assert wrel in seen, (b_i, wrel)
        mm_base.append(col)
        col += len(entries)
        sched.append(entries)
    n_mm_tot = col

    # per-core input tensors
    in_maps = []
    for ci in range(c):
        s, e = core_start[ci], core_start[ci + 1]
        sl = slot[s:e]
        gt = gt_all[s:e]
        p_e = sl % P
        idx_w = np.zeros((P, ntile_tot * 8), dtype=np.int16)
        colb = tile_base[b_s[s:e], g_s[s:e]] * 8
        cr = (col_s[s:e] - g_s[s:e] * GROUP_ROWS).astype(np.int16)
        ccol = colb + sl // 16
        crow = (sl % 16).astype(np.int64)
        for k in range(8):
            idx_w[crow + 16 * k, ccol] = cr
        # edge values pre-expanded along the feature dim so the on-chip
        # scale op has packed operands (DVE 2x mode)
        vv = np.zeros((P, ntile_tot), dtype=NP_BF16)
        vv[p_e, gt] = vv_s[s:e].astype(NP_BF16)
        vexp = np.ascontiguousarray(
            np.broadcast_to(vv[:, :, None], (P, ntile_tot, in_f))
        ).reshape(P, ntile_tot * in_f)
        rl = np.full((P, n_mm_tot), RL_PAD, dtype=NP_BF16)
        mm_col = col_map[gt, w_s[s:e] - lo[gt]]
        rl[p_e, mm_col] = d_s[s:e].astype(NP_BF16)
        in_maps.append(
            dict(xb=xb, wt=wt, biasT=biasT, iota=iota, gidx=idx_w, rloc=rl, vals=vexp)
        )

    shared = dict(
        T=T,
        tile_base=tile_base,
        ntile_tot=ntile_tot,
        batch_tiles=batch_tiles,
        batch_tile0=batch_tile0,
        sched=sched,
        mm_base=mm_base,
        n_mm_tot=n_mm_tot,
        nb=nb,
        ngrp=ngrp,
    )
    return in_maps, shared


def build(nc, shared, cfg):
    """Trace the (per-core identical) kernel program."""
    out_f = cfg["out_f"]
    in_f = cfg["in_f"]
    gw = cfg["gw"]
    sc = cfg["sc"]
    nq = cfg["nq"]
    ns, nw, ntab, npad = _derived(cfg)
    assert in_f == P
    nb = shared["nb"]
    ngrp = shared["ngrp"]
    T = shared["T"]
    tile_base = shared["tile_base"]
    ntile_tot = shared["ntile_tot"]
    batch_tiles = shared["batch_tiles"]
    batch_tile0 = shared["batch_tile0"]
    sched = shared["sched"]
    mm_base = shared["mm_base"]
    n_mm_tot = shared["n_mm_tot"]
    max_bt = int(batch_tiles.max())
    max_bm = max(len(s) for s in sched)

    xb_d = nc.dram_tensor("xb", [npad, in_f], BF16, kind="ExternalInput")
    wt_d = nc.dram_tensor("wt", [P, out_f], BF16, kind="ExternalInput")
    biasT_d = nc.dram_tensor("biasT", [out_f, 1], F32, kind="ExternalInput")
    iota_d = nc.dram_tensor("iota", [P, P], BF16, kind="ExternalInput")
    gidx_d = nc.dram_tensor("gidx", [P, ntile_tot * 8], I16, kind="ExternalInput")
    rloc_d = nc.dram_tensor("rloc", [P, n_mm_tot], BF16, kind="ExternalInput")
    vals_d = nc.dram_tensor("vals", [P, ntile_tot * in_f], BF16, kind="ExternalInput")
    out_d = nc.dram_tensor("out", [out_f, nw * P], F32, kind="ExternalOutput")

    eq = mybir.AluOpType.is_equal
    mul = mybir.AluOpType.mult
    add = mybir.AluOpType.add

    qn = [0]

    with tile.TileContext(nc) as tc:
        with (
            tc.tile_pool(name="const", bufs=1) as cpool,
            tc.tile_pool(name="edges", bufs=2) as epool,
            tc.tile_pool(name="gbuf", bufs=3) as gpool,
            tc.tile_pool(name="smat", bufs=2) as spool,
            tc.tile_pool(name="apsum", bufs=2, space="PSUM") as appool,
            tc.tile_pool(name="aggT", bufs=3) as atpool,
            tc.tile_pool(name="ppsum", bufs=2, space="PSUM") as prpool,
            tc.tile_pool(name="ot", bufs=2) as opool,
        ):
            wt_t = cpool.tile([P, out_f], BF16)
            nc.sync.dma_start(out=wt_t[:], in_=wt_d[:])
            iota_t = cpool.tile([P, P], BF16)
            nc.sync.dma_start(out=iota_t[:], in_=iota_d[:])
            biasT_t = cpool.tile([out_f, 1], F32)
            nc.sync.dma_start(out=biasT_t[:], in_=biasT_d[:])

            for b in range(nb):
                bt = int(batch_tiles[b])
                t0 = int(batch_tile0[b])
                entries = sched[b]
                bm = len(entries)
                m0 = mm_base[b]
                gwb = min(gw, nw - b * gw)

                idx_t = epool.tile([P, max_bt * 8], I16, tag="idx")
                rl_t = epool.tile([P, max_bm], BF16, tag="rl")
                vexp_t = epool.tile([P, max_bt * in_f], BF16, tag="vexp")
                nc.scalar.dma_start(
                    out=idx_t[:, : bt * 8], in_=gidx_d[:, t0 * 8 : (t0 + bt) * 8]
                )
                nc.scalar.dma_start(out=rl_t[:, :bm], in_=rloc_d[:, m0 : m0 + bm])
                nc.scalar.dma_start(
                    out=vexp_t[:, : bt * in_f],
                    in_=vals_d[:, t0 * in_f : (t0 + bt) * in_f],
                )

                # batched gathers of raw 256B x rows, <=512 idxs per ucode call
                gb = gpool.tile([P, max_bt * in_f], BF16, tag="gb")
                for g in range(ngrp):
                    tg = int(T[b, g])
                    if tg == 0:
                        continue
                    tb = int(tile_base[b, g]) - t0
                    r0 = g * GROUP_ROWS
                    r1 = min((g + 1) * GROUP_ROWS, npad)
                    for cq in range(0, tg, CALL_TILES):
                        cn = min(CALL_TILES, tg - cq)
                        ta = tb + cq
                        nc.gpsimd.dma_gather(
                            out_ap=gb[:, ta * in_f : (ta + cn) * in_f].rearrange(
                                "p (t f) -> p t f", f=in_f
                            ),
                            in_ap=xb_d[r0:r1, :],
                            idxs_ap=idx_t[:, ta * 8 : (ta + cn) * 8],
                            num_idxs=cn * P,
                            num_idxs_reg=cn * P,
                            elem_size=in_f,
                            queue_num=qn[0],
                        )
                        qn[0] = (qn[0] + 1) % nq
                # scale gathered rows by edge values in place (packed bf16
                # operands keep DVE in 2x mode)
                nc.vector.tensor_tensor(
                    out=gb[:, : bt * in_f],
                    in0=gb[:, : bt * in_f],
                    in1=vexp_t[:, : bt * in_f],
                    op=mul,
                )

                # scatter: aggT[k, d] += sum_slots G[slot, k] * S[slot, d]
                aggT_ps = appool.tile([P, gw * P], F32, tag="aggT_ps")
                for c0 in range(0, bm, sc):
                    cn = min(sc, bm - c0)
                    smat = spool.tile([P, sc * P], BF16, tag="S")
                    nc.vector.tensor_tensor(
                        out=smat[:, : cn * P].rearrange("p (m d) -> p m d", d=P),
                        in0=iota_t[:].unsqueeze(1).broadcast_to([P, cn, P]),
                        in1=rl_t[:, c0 : c0 + cn]
                        .unsqueeze(2)
                        .broadcast_to([P, cn, P]),
                        op=eq,
                    )
                    for i in range(cn):
                        t_b, wrel, mst, msp = entries[c0 + i]
                        nc.tensor.matmul(
                            out=aggT_ps[:, wrel * P : (wrel + 1) * P],
                            lhsT=gb[:, t_b * in_f : (t_b + 1) * in_f],
                            rhs=smat[:, i * P : (i + 1) * P],
                            start=mst,
                            stop=msp,
                        )
                # project finished windows in groups of 3: outT = W.T @ aggT,
                # + bias (evac + bias-add ride the idle Activation engine)
                ot = opool.tile([out_f, gw * P], F32, tag="ot")
                for w0 in range(0, gwb, 3):
                    wn = min(3, gwb - w0)
                    aggT_sb = atpool.tile([P, 3 * P], BF16, tag="aggT_sb")
                    nc.scalar.copy(
                        out=aggT_sb[:, : wn * P],
                        in_=aggT_ps[:, w0 * P : (w0 + wn) * P],
                    )
                    pr_ps = prpool.tile([out_f, 3 * P], F32, tag="pr")
                    nc.tensor.matmul(
                        out=pr_ps[:, : wn * P],
                        lhsT=wt_t[:],
                        rhs=aggT_sb[:, : wn * P],
                        start=True,
                        stop=True,
                    )
                    nc.scalar.activation(
                        out=ot[:, w0 * P : (w0 + wn) * P],
                        in_=pr_ps[:, : wn * P],
                        func=mybir.ActivationFunctionType.Identity,
                        bias=biasT_t[:],
                    )
                nc.sync.dma_start(
                    out=out_d[:, b * gw * P : (b * gw + gwb) * P],
                    in_=ot[:, : gwb * P],
                )
    return nc


def assemble_output(results, cfg):
    out_f = cfg["out_f"]
    ns, nw, ntab, npad = _derived(cfg)
    blocks = []
    for r in results:
        o = np.asarray(r["out"], dtype=np.float32)  # [out_f, nw*P]
        o = o.reshape(out_f, nw * P).T[:ns]  # [ns, out_f]
        blocks.append(o)
    return np.ascontiguousarray(np.concatenate(blocks, axis=0))


LAST_RESULTS = None


def kernel(x, weights, bias, adj_rows, adj_cols, adj_vals):
    global LAST_RESULTS
    cfg = default_cfg()
    in_maps, shared = prep_inputs(x, weights, bias, adj_rows, adj_cols, adj_vals, cfg)
    nc = bacc.Bacc(
        "TRN2",
        target_bir_lowering=False,
        debug=False,
        num_swdge_queues=cfg["nq"],
        dynamic_dma_scratch_size=32768,
    )
    build(nc, shared, cfg)
    nc.compile()
    res = None
    for attempt in range(3):
        try:
            res = bass_utils.run_bass_kernel_spmd(
                nc, in_maps, core_ids=list(range(cfg["n_cores"]))
            )
            break
        except Exception:
            # an earlier run can leave the exec unit wedged; a retry
            # (which triggers a device reset) normally recovers
            if attempt == 2:
                raise
    LAST_RESULTS = res
    return assemble_output(res.results, cfg)


# revision 33
# speedup vs baseline: 1.0267x; 1.0267x over previous
"""GCN layer (x@W, sparse-adj aggregate, +bias) on 8 Trainium2 NeuronCores.

Strategy (memory-regime), aggregate-then-project:
  out = A @ (x @ W) + b == (A @ x) @ W + b

  - Destination nodes sharded 12500/core (1D graph partition per hint).
  - Edges bucketed by (window-batch of gw dest windows, int16 index group
    of 32768 source rows), sorted by dest window within each bucket,
    padded to 128-slot tiles (host-side, shared tile counts across cores).
  - The gather reads RAW x rows (256B bf16) straight from the input
    table in DRAM via batched dma_gather ucode calls (<=512 idxs/call,
    round-robin over 4 SWDGE queues, 32KB descriptor scratch) -- no
    projection table is materialized, so gathering starts immediately.
  - Gathered rows are scaled in place by edge values (broadcast
    tensor_tensor); one-hot scatter matrices S for chunks of matmuls are
    built with broadcast is_equal tensor_tensor ops; PE matmuls
    G_t.T @ S accumulate agg^T per 128-row dest window in PSUM.
  - agg^T windows are evacuated to SBUF bf16 and projected by W
    (lhsT=W, rhs=agg^T) into out^T; bias is added per-partition on PSUM
    evacuation; one streaming DMA per batch writes out^T; the host
    transposes back.
"""

import math
import sys

import numpy as np

for _p in ("/opt/trn_rl_repo",):
    if _p not in sys.path:
        sys.path.insert(0, _p)

import ml_dtypes  # noqa: E402

from concourse import bacc, bass, mybir, tile  # noqa: E402
from concourse import bass_utils  # noqa: E402

BF16 = mybir.dt.bfloat16
F32 = mybir.dt.float32
I16 = mybir.dt.int16
NP_BF16 = ml_dtypes.bfloat16

P = 128
GROUP_ROWS = 32768  # int16 index reach of dma_gather
CALL_TILES = 4  # ucode caps one dma_gather at 512 idxs = 4 tiles
RL_PAD = 255.0  # rloc sentinel that never matches iota (0..127)


def default_cfg():
    return dict(
        n_nodes=100000,
        n_edges=800000,
        in_f=128,
        out_f=64,
        n_cores=8,
        gw=12,  # dest windows per batch (PSUM: 12*512B = 3 banks per buf)
        sc=32,  # matmuls per S-matrix build chunk
        nq=4,  # SWDGE queues for gather descriptor rings
    )


def _derived(cfg):
    n_nodes = cfg["n_nodes"]
    c = cfg["n_cores"]
    ns = n_nodes // c  # dest rows per core
    nw = math.ceil(ns / P)  # dest windows per core
    ntab = math.ceil(n_nodes / P)  # source row tiles
    npad = ntab * P
    return ns, nw, ntab, npad


def prep_inputs(x, weights, bias, adj_rows, adj_cols, adj_vals, cfg):
    """Host-side sharding/index prep (numpy only). Returns (in_maps, shared)."""
    c = cfg["n_cores"]
    in_f = cfg["in_f"]
    gw = cfg["gw"]
    ns, nw, ntab, npad = _derived(cfg)
    nb = math.ceil(nw / gw)
    ngrp = math.ceil(npad / GROUP_ROWS)

    x = np.asarray(x, dtype=np.float32)
    weights = np.asarray(weights, dtype=np.float32)
    bias = np.asarray(bias, dtype=np.float32)
    rows = np.asarray(adj_rows).astype(np.int64)
    cols = np.asarray(adj_cols).astype(np.int64)
    vals = np.asarray(adj_vals, dtype=np.float32)

    xb = np.zeros((npad, in_f), dtype=NP_BF16)
    xb[: x.shape[0]] = x.astype(NP_BF16)
    wt = weights.astype(NP_BF16)
    biasT = np.ascontiguousarray(bias[:, None]).astype(np.float32)  # [64, 1]
    iota = np.broadcast_to(np.arange(P, dtype=np.float32), (P, P)).astype(NP_BF16)
    iota = np.ascontiguousarray(iota)

    # per-edge attributes
    core = rows // ns
    rloc = rows - core * ns
    w = rloc // P
    d = rloc % P
    b = w // gw
    g = cols // GROUP_ROWS

    # sort by (core, batch, group, window)
    key = ((core * nb + b) * ngrp + g) * nw + w
    order = np.argsort(key, kind="stable")
    core_s = core[order]
    b_s = b[order]
    g_s = g[order]
    w_s = w[order]
    d_s = d[order]
    col_s = cols[order]
    vv_s = vals[order]

    # bucket counts n[core, b, g]
    bg = b_s * ngrp + g_s
    cnt = np.zeros((c, nb * ngrp), dtype=np.int64)
    for ci in range(c):
        m = core_s == ci
        cnt[ci] = np.bincount(bg[m], minlength=nb * ngrp)
    cnt = cnt.reshape(c, nb, ngrp)
    T = -(-cnt.max(axis=0) // P)  # [nb, ngrp] shared tile counts

    tile_base = np.zeros((nb, ngrp), dtype=np.int64)
    np.cumsum(T.ravel()[:-1], out=tile_base.ravel()[1:])
    ntile_tot = int(T.sum())
    batch_tiles = T.sum(axis=1)  # tiles per batch
    batch_tile0 = np.concatenate([[0], np.cumsum(batch_tiles)[:-1]])

    # per-core slot assignment within each (b, g) bucket
    core_start = np.searchsorted(core_s, np.arange(c + 1))
    slot = np.zeros(len(order), dtype=np.int64)  # local slot within bucket
    for ci in range(c):
        s, e = core_start[ci], core_start[ci + 1]
        bgl = bg[s:e]
        bstart = np.searchsorted(bgl, np.arange(nb * ngrp))
        slot[s:e] = np.arange(e - s) - bstart[bgl]

    # shared matmul schedule: per (b, g, tile): union window span over cores
    lo = np.full((ntile_tot,), 1 << 30, dtype=np.int64)
    hi = np.full((ntile_tot,), -1, dtype=np.int64)
    gt_all = tile_base[b_s, g_s] + slot // P  # global tile per edge
    np.minimum.at(lo, gt_all, w_s)
    np.maximum.at(hi, gt_all, w_s)
    # tiles with no edges anywhere: one dummy matmul at the batch's first
    # window (S will be all-zero)
    for b_i in range(nb):
        for g_i in range(ngrp):
            for t_l in range(T[b_i, g_i]):
                gt = tile_base[b_i, g_i] + t_l
                if hi[gt] < 0:
                    lo[gt] = hi[gt] = b_i * gw
    span = hi - lo + 1
    # presence[gt, wv-lo]: does ANY core have an edge in (tile, window)?
    # Scheduling only present pairs trims union-span middles with no edges.
    presence = np.zeros((ntile_tot, int(span.max())), dtype=bool)
    presence[gt_all, w_s - lo[gt_all]] = True
    for b_i in range(nb):
        for g_i in range(ngrp):
            for t_l in range(T[b_i, g_i]):
                gt = tile_base[b_i, g_i] + t_l
                if not presence[gt, : span[gt]].any():
                    presence[gt, 0] = True  # dummy (all-zero S)

    # per-batch schedule entries (tile_in_batch, wrel, start, stop), ordered
    # window-major so only ONE PSUM accumulation group is open at a time
    # (PSUM groups conflict at bank granularity). col_map[(gt, wv-lo)] gives
    # each edge its rl column under the final order.
    col_map = np.zeros((ntile_tot, int(span.max())), dtype=np.int64)
    sched = []
    mm_base = []
    col = 0
    for b_i in range(nb):
        raw = []
        for g_i in range(ngrp):
            for t_l in range(T[b_i, g_i]):
                gt = tile_base[b_i, g_i] + t_l
                for wv in range(lo[gt], hi[gt] + 1):
                    if presence[gt, wv - lo[gt]]:
                        raw.append((wv - b_i * gw, gt))
        raw.sort()
        entries = []
        gwb = min(gw, nw - b_i * gw)
        seen = set()
        for i, (wrel, gt) in enumerate(raw):
            first = wrel not in seen
            seen.add(wrel)
            last = i + 1 == len(raw) or raw[i + 1][0] != wrel
            entries.append([gt - batch_tile0[b_i], wrel, first, last])
            col_map[gt, wrel + b_i * gw - lo[gt]] = col + i
        for wrel in range(gwb):
            assert wrel in seen, (b_i, wrel)
        mm_base.append(col)
        col += len(entries)
        sched.append(entries)
    n_mm_tot = col

    # per-core input tensors
    in_maps = []
    for ci in range(c):
        s, e = core_start[ci], core_start[ci + 1]
        sl = slot[s:e]
        gt = gt_all[s:e]
        p_e = sl % P
        idx_w = np.zeros((P, ntile_tot * 8), dtype=np.int16)
        colb = tile_base[b_s[s:e], g_s[s:e]] * 8
        cr = (col_s[s:e] - g_s[s:e] * GROUP_ROWS).astype(np.int16)
        ccol = colb + sl // 16
        crow = (sl % 16).astype(np.int64)
        for k in range(8):
            idx_w[crow + 16 * k, ccol] = cr
        # edge values pre-expanded 16-wide so the on-chip scale op keeps a
        # packed last dim (DVE 2x mode) while the DMA stays small
        vv = np.zeros((P, ntile_tot), dtype=NP_BF16)
        vv[p_e, gt] = vv_s[s:e].astype(NP_BF16)
        vexp = np.ascontiguousarray(
            np.broadcast_to(vv[:, :, None], (P, ntile_tot, 16))
        ).reshape(P, ntile_tot * 16)
        rl = np.full((P, n_mm_tot), RL_PAD, dtype=NP_BF16)
        mm_col = col_map[gt, w_s[s:e] - lo[gt]]
        rl[p_e, mm_col] = d_s[s:e].astype(NP_BF16)
        in_maps.append(
            dict(xb=xb, wt=wt, biasT=biasT, iota=iota, gidx=idx_w, rloc=rl, vals=vexp)
        )

    shared = dict(
        T=T,
        tile_base=tile_base,
        ntile_tot=ntile_tot,
        batch_tiles=batch_tiles,
        batch_tile0=batch_tile0,
        sched=sched,
        mm_base=mm_base,
        n_mm_tot=n_mm_tot,
        nb=nb,
        ngrp=ngrp,
    )
    return in_maps, shared


def build(nc, shared, cfg):
    """Trace the (per-core identical) kernel program."""
    out_f = cfg["out_f"]
    in_f = cfg["in_f"]
    gw = cfg["gw"]
    sc = cfg["sc"]
    nq = cfg["nq"]
    ns, nw, ntab, npad = _derived(cfg)
    assert in_f == P
    nb = shared["nb"]
    ngrp = shared["ngrp"]
    T = shared["T"]
    tile_base = shared["tile_base"]
    ntile_tot = shared["ntile_tot"]
    batch_tiles = shared["batch_tiles"]
    batch_tile0 = shared["batch_tile0"]
    sched = shared["sched"]
    mm_base = shared["mm_base"]
    n_mm_tot = shared["n_mm_tot"]
    max_bt = int(batch_tiles.max())
    max_bm = max(len(s) for s in sched)

    xb_d = nc.dram_tensor("xb", [npad, in_f], BF16, kind="ExternalInput")
    wt_d = nc.dram_tensor("wt", [P, out_f], BF16, kind="ExternalInput")
    biasT_d = nc.dram_tensor("biasT", [out_f, 1], F32, kind="ExternalInput")
    iota_d = nc.dram_tensor("iota", [P, P], BF16, kind="ExternalInput")
    gidx_d = nc.dram_tensor("gidx", [P, ntile_tot * 8], I16, kind="ExternalInput")
    rloc_d = nc.dram_tensor("rloc", [P, n_mm_tot], BF16, kind="ExternalInput")
    vals_d = nc.dram_tensor("vals", [P, ntile_tot * 16], BF16, kind="ExternalInput")
    out_d = nc.dram_tensor("out", [out_f, nw * P], BF16, kind="ExternalOutput")

    eq = mybir.AluOpType.is_equal
    mul = mybir.AluOpType.mult
    add = mybir.AluOpType.add

    qn = [0]

    with tile.TileContext(nc) as tc:
        with (
            tc.tile_pool(name="const", bufs=1) as cpool,
            tc.tile_pool(name="edges", bufs=2) as epool,
            tc.tile_pool(name="gbuf", bufs=3) as gpool,
            tc.tile_pool(name="smat", bufs=2) as spool,
            tc.tile_pool(name="apsum", bufs=2, space="PSUM") as appool,
            tc.tile_pool(name="aggT", bufs=3) as atpool,
            tc.tile_pool(name="ppsum", bufs=2, space="PSUM") as prpool,
            tc.tile_pool(name="ot", bufs=2) as opool,
        ):
            wt_t = cpool.tile([P, out_f], BF16)
            nc.sync.dma_start(out=wt_t[:], in_=wt_d[:])
            iota_t = cpool.tile([P, P], BF16)
            nc.sync.dma_start(out=iota_t[:], in_=iota_d[:])
            biasT_t = cpool.tile([out_f, 1], F32)
            nc.sync.dma_start(out=biasT_t[:], in_=biasT_d[:])

            for b in range(nb):
                bt = int(batch_tiles[b])
                t0 = int(batch_tile0[b])
                entries = sched[b]
                bm = len(entries)
                m0 = mm_base[b]
                gwb = min(gw, nw - b * gw)

                idx_t = epool.tile([P, max_bt * 8], I16, tag="idx")
                rl_t = epool.tile([P, max_bm], BF16, tag="rl")
                vexp_t = epool.tile([P, max_bt * 16], BF16, tag="vexp")
                nc.scalar.dma_start(
                    out=idx_t[:, : bt * 8], in_=gidx_d[:, t0 * 8 : (t0 + bt) * 8]
                )
                nc.scalar.dma_start(out=rl_t[:, :bm], in_=rloc_d[:, m0 : m0 + bm])
                nc.scalar.dma_start(
                    out=vexp_t[:, : bt * 16],
                    in_=vals_d[:, t0 * 16 : (t0 + bt) * 16],
                )

                # batched gathers of raw 256B x rows, <=512 idxs per ucode call
                gb = gpool.tile([P, max_bt * in_f], BF16, tag="gb")
                for g in range(ngrp):
                    tg = int(T[b, g])
                    if tg == 0:
                        continue
                    tb = int(tile_base[b, g]) - t0
                    r0 = g * GROUP_ROWS
                    r1 = min((g + 1) * GROUP_ROWS, npad)
                    for cq in range(0, tg, CALL_TILES):
                        cn = min(CALL_TILES, tg - cq)
                        ta = tb + cq
                        nc.gpsimd.dma_gather(
                            out_ap=gb[:, ta * in_f : (ta + cn) * in_f].rearrange(
                                "p (t f) -> p t f", f=in_f
                            ),
                            in_ap=xb_d[r0:r1, :],
                            idxs_ap=idx_t[:, ta * 8 : (ta + cn) * 8],
                            num_idxs=cn * P,
                            num_idxs_reg=cn * P,
                            elem_size=in_f,
                            queue_num=qn[0],
                        )
                        qn[0] = (qn[0] + 1) % nq
                # scale gathered rows by edge values in place; the 16-wide
                # vexp replica is broadcast on a middle axis so every
                # operand's LAST dim stays packed (DVE 2x mode)
                nc.vector.tensor_tensor(
                    out=gb[:, : bt * in_f].rearrange(
                        "p (t a b) -> p t a b", a=in_f // 16, b=16
                    ),
                    in0=gb[:, : bt * in_f].rearrange(
                        "p (t a b) -> p t a b", a=in_f // 16, b=16
                    ),
                    in1=vexp_t[:, : bt * 16]
                    .rearrange("p (t b) -> p t b", b=16)
                    .unsqueeze(2)
                    .broadcast_to([P, bt, in_f // 16, 16]),
                    op=mul,
                )

                # scatter: aggT[k, d] += sum_slots G[slot, k] * S[slot, d]
                aggT_ps = appool.tile([P, gw * P], F32, tag="aggT_ps")
                for c0 in range(0, bm, sc):
                    cn = min(sc, bm - c0)
                    smat = spool.tile([P, sc * P], BF16, tag="S")
                    nc.vector.tensor_tensor(
                        out=smat[:, : cn * P].rearrange("p (m d) -> p m d", d=P),
                        in0=iota_t[:].unsqueeze(1).broadcast_to([P, cn, P]),
                        in1=rl_t[:, c0 : c0 + cn]
                        .unsqueeze(2)
                        .broadcast_to([P, cn, P]),
                        op=eq,
                    )
                    for i in range(cn):
                        t_b, wrel, mst, msp = entries[c0 + i]
                        nc.tensor.matmul(
                            out=aggT_ps[:, wrel * P : (wrel + 1) * P],
                            lhsT=gb[:, t_b * in_f : (t_b + 1) * in_f],
                            rhs=smat[:, i * P : (i + 1) * P],
                            start=mst,
                            stop=msp,
                        )
                # project finished windows in groups of 3: outT = W.T @ aggT,
                # + bias (evac + bias-add ride the idle Activation engine)
                ot = opool.tile([out_f, gw * P], BF16, tag="ot")
                for w0 in range(0, gwb, 3):
                    wn = min(3, gwb - w0)
                    aggT_sb = atpool.tile([P, 3 * P], BF16, tag="aggT_sb")
                    nc.scalar.copy(
                        out=aggT_sb[:, : wn * P],
                        in_=aggT_ps[:, w0 * P : (w0 + wn) * P],
                    )
                    pr_ps = prpool.tile([out_f, 3 * P], F32, tag="pr")
                    nc.tensor.matmul(
                        out=pr_ps[:, : wn * P],
                        lhsT=wt_t[:],
                        rhs=aggT_sb[:, : wn * P],
                        start=True,
                        stop=True,
                    )
                    nc.scalar.activation(
                        out=ot[:, w0 * P : (w0 + wn) * P],
                        in_=pr_ps[:, : wn * P],
                        func=mybir.ActivationFunctionType.Identity,
                        bias=biasT_t[:],
                    )
                nc.sync.dma_start(
                    out=out_d[:, b * gw * P : (b * gw + gwb) * P],
                    in_=ot[:, : gwb * P],
                )
    return nc


def assemble_output(results, cfg):
    out_f = cfg["out_f"]
    ns, nw, ntab, npad = _derived(cfg)
    blocks = []
    for r in results:
        o = np.asarray(r["out"], dtype=np.float32)  # [out_f, nw*P]
        o = o.reshape(out_f, nw * P).T[:ns]  # [ns, out_f]
        blocks.append(o)
    return np.ascontiguousarray(np.concatenate(blocks, axis=0))


LAST_RESULTS = None


def kernel(x, weights, bias, adj_rows, adj_cols, adj_vals):
    global LAST_RESULTS
    cfg = default_cfg()
    in_maps, shared = prep_inputs(x, weights, bias, adj_rows, adj_cols, adj_vals, cfg)
    nc = bacc.Bacc(
        "TRN2",
        target_bir_lowering=False,
        debug=False,
        num_swdge_queues=cfg["nq"],
        dynamic_dma_scratch_size=32768,
    )
    build(nc, shared, cfg)
    nc.compile()
    res = None
    for attempt in range(3):
        try:
            res = bass_utils.run_bass_kernel_spmd(
                nc, in_maps, core_ids=list(range(cfg["n_cores"]))
            )
            break
        except Exception:
            # an earlier run can leave the exec unit wedged; a retry
            # (which triggers a device reset) normally recovers
            if attempt == 2:
                raise
    LAST_RESULTS = res
    return assemble_output(res.results, cfg)
